# revision 1
# baseline (speedup 1.0000x reference)
"""Trainium2 Bass kernel for nn_GATModel (2-layer GAT + mean-pool + MLP head).

v2 strategy (8 NeuronCores, SPMD, dst-sharded):
  - Edges sorted by dst; each core owns a contiguous 6250-node range and all
    edges pointing into it. 49 windows of 128 dst nodes; per-window chunk
    count k_w = max over cores (shared SPMD program shape).
  - Per layer, a per-node table row [z_h|1|0]*H | el(3) | er(3) | pad (bf16)
    is computed by PE matmuls on the own shard and AllGathered; indirect
    gathers read the Shared output directly.
  - Edge phase per window: ONE batched indirect gather (offset AP [128,k])
    pulls all k*128 src rows; host-precomputed one-hot masks (maskT for
    er-expand node->edge, wm0 for aggregation) are DMA'd; e = el+er on DVE
    (window-wide), w = exp(lrelu(e)) on ACT; gathered z blocks scaled by w
    in place (3 small DVE ops per chunk); ONE aggregation matmul per chunk
    accumulating [128, 3*(D+2)] in PSUM; softmax denominator via scaled
    ones-columns.
  - er values for own nodes stay SBUF-resident ([128, 4*NWIN]).
  - Mean-pool fused into layer-2 epilogue (one-hot graph masks + 1/cnt),
    AllReduce [192,128], dense head on PE.
"""
import math
import os
from contextlib import ExitStack

import numpy as np
import ml_dtypes

import concourse.bacc as bacc
import concourse.bass as bass
import concourse.tile as tile
from concourse import mybir
from concourse.bass_utils import run_bass_kernel_spmd

dt = mybir.dt

N_NODES = 50000
N_EDGES = 800000
N_GRAPHS = 128
NEG = 0.2
NC = 8
NPC = N_NODES // NC            # 6250 nodes per core
NWIN = math.ceil(NPC / 128)    # 49 windows per core
H = 3
D1, D2 = 32, 64
B1, B2 = D1 + 2, D2 + 2        # head block: z | one | pad
ZC1, ZC2 = H * B1, H * B2      # 102, 198
R1 = 128                       # [34*3=102 z | el 3 | er 3 | pad] -> 256B rows
R2 = 256                       # [66*3=198 z | el 3 | er 3 | pad] -> 512B rows
NLO = 32768                    # int16 index limit: src < NLO in lo gather
EL1, ER1 = ZC1, ZC1 + 3
EL2, ER2 = ZC2, ZC2 + 3
X1C, X2C = 96, 192

_CACHE = {}


def _ceil(a, b):
    return (a + b - 1) // b


def build_program(kws):
    """kws: per-window chunk counts (shared across cores)."""
    STAGE = int(os.environ.get("K_STAGE", "99"))
    M8 = int(os.environ.get("K_MASK8", "1"))
    mdt = dt.float8e4 if M8 else dt.bfloat16
    NW_LIM = int(os.environ.get("K_NWIN", str(NWIN)))
    nc = bacc.Bacc("TRN2", target_bir_lowering=False, debug=False, num_devices=NC,
                   dynamic_dma_scratch_size=49152, num_swdge_queues=4)
    kws = list(kws)              # [(kA, kB)] per window
    ktot = [a + b for a, b in kws]
    TCH = sum(ktot)
    KMAX = max(ktot)
    K0s = np.concatenate([[0], np.cumsum(ktot)]).astype(int)
    ECOLS = TCH * 128

    # ---------------- I/O ----------------
    featT = nc.dram_tensor("featT", [11, NPC], dt.float32, kind="ExternalInput").ap()
    wcat1 = nc.dram_tensor("wcat1", [11, R1], dt.float32, kind="ExternalInput").ap()
    wcat2 = nc.dram_tensor("wcat2", [X1C + 1, R2], dt.bfloat16, kind="ExternalInput").ap()
    idx_in = nc.dram_tensor("idx16", [128, 8 * TCH], dt.int16, kind="ExternalInput").ap()
    maskT_in = nc.dram_tensor("maskT", [128, ECOLS], mdt, kind="ExternalInput").ap()
    wm0_in = nc.dram_tensor("wm0", [128, ECOLS], mdt, kind="ExternalInput").ap()
    gidc_in = nc.dram_tensor("gidc", [128, NWIN], dt.float32, kind="ExternalInput").ap()
    invc_in = nc.dram_tensor("invc", [128, NWIN], dt.float32, kind="ExternalInput").ap()
    d1a_in = nc.dram_tensor("d1a", [128, 64], dt.float32, kind="ExternalInput").ap()
    d1b_in = nc.dram_tensor("d1b", [65, 64], dt.float32, kind="ExternalInput").ap()
    d2_in = nc.dram_tensor("d2", [65, 1], dt.float32, kind="ExternalInput").ap()
    ident_in = nc.dram_tensor("ident", [128, 128], dt.bfloat16, kind="ExternalInput").ap()
    iota_in = nc.dram_tensor("iota_row", [128, 128], dt.bfloat16, kind="ExternalInput").ap()
    out_ext = nc.dram_tensor("out", [N_GRAPHS, 1], dt.float32, kind="ExternalOutput").ap()

    rg = [list(range(NC))]

    with tile.TileContext(nc) as tc, ExitStack() as ctx:
        cst = ctx.enter_context(tc.tile_pool(name="cst", bufs=1))
        sb = ctx.enter_context(tc.tile_pool(name="sb", bufs=2))
        dr = ctx.enter_context(tc.tile_pool(name="dr", bufs=1, space="DRAM"))
        psA = ctx.enter_context(tc.tile_pool(name="psA", bufs=2, space="PSUM"))

        # ---------------- constants / resident ----------------
        ident = cst.tile([128, 128], dt.bfloat16)
        nc.sync.dma_start(out=ident[:], in_=ident_in)
        iota_row = cst.tile([128, 128], dt.bfloat16)
        nc.sync.dma_start(out=iota_row[:], in_=iota_in)

        idx16 = cst.tile([128, 8 * TCH], dt.int16)
        nc.sync.dma_start(out=idx16[:], in_=idx_in)
        gidc = cst.tile([128, NWIN], dt.float32)
        nc.sync.dma_start(out=gidc[:], in_=gidc_in)
        invc = cst.tile([128, NWIN], dt.float32)
        nc.sync.dma_start(out=invc[:], in_=invc_in)
        w1sb = cst.tile([11, R1], dt.float32)
        nc.sync.dma_start(out=w1sb[:], in_=wcat1)
        w2sb = cst.tile([X1C + 1, R2], dt.bfloat16)
        nc.sync.dma_start(out=w2sb[:], in_=wcat2)
        ftsb = cst.tile([11, NPC], dt.float32)
        nc.sync.dma_start(out=ftsb[:], in_=featT)
        d1a = cst.tile([128, 64], dt.float32)
        nc.sync.dma_start(out=d1a[:], in_=d1a_in)
        d1b = cst.tile([65, 64], dt.float32)
        nc.sync.dma_start(out=d1b[:], in_=d1b_in)
        d2w = cst.tile([65, 1], dt.float32)
        nc.sync.dma_start(out=d2w[:], in_=d2_in)

        er1_sb = cst.tile([128, 4 * NWIN], dt.bfloat16)
        nc.vector.memset(er1_sb[:], 0.0)
        er2_sb = cst.tile([128, 4 * NWIN], dt.bfloat16)
        nc.vector.memset(er2_sb[:], 0.0)

        t1_shard = dr.tile([NPC, R1], dt.bfloat16)
        t2_shard = dr.tile([NPC, R2], dt.bfloat16)
        pool_loc = dr.tile([X2C, N_GRAPHS], dt.float32)
        table1_t = dr.tile([N_NODES, R1], dt.bfloat16, addr_space="Shared", name="table1_t")
        table2_t = dr.tile([N_NODES, R2], dt.bfloat16, addr_space="Shared", name="table2_t")
        pool_red_t = dr.tile([X2C, N_GRAPHS], dt.float32, addr_space="Shared", name="pool_red_t")

        # ---------------- phase 0 + layer-1 edge phase share psB ----------------
        with tc.tile_pool(name="psB", bufs=2, space="PSUM") as psB:
            # phase 0: layer-1 table shard
            for w in range(NWIN):
                n0 = w * 128
                nw = min(128, NPC - n0)
                zp = psB.tile([128, R2], dt.float32, tag="zbig", name="zp")
                nc.tensor.matmul(zp[:nw, :R1], lhsT=ftsb[:, n0:n0 + nw], rhs=w1sb[:],
                                 start=True, stop=True)
                zb = sb.tile([128, R1], dt.bfloat16, tag="zb1", name="zb1")
                nc.vector.tensor_copy(zb[:nw, :], zp[:nw, :R1])
                nc.vector.tensor_copy(er1_sb[:nw, 4 * w:4 * w + 3],
                                      zp[:nw, ER1:ER1 + 3])
                nc.sync.dma_start(out=t1_shard[n0:n0 + nw, :], in_=zb[:nw, :])

            nc.gpsimd.collective_compute(
                "AllGather", mybir.AluOpType.bypass, replica_groups=rg,
                ins=[t1_shard[:, :]], outs=[table1_t[:, :]])

            if STAGE >= 1:
                edge_layer(nc, tc, sb, psA, psB, None, 1,
                           kws[:NW_LIM], K0s,
                           table1_t, idx16, maskT_in, wm0_in, er1_sb, er2_sb,
                           ident, iota_row, gidc, invc, w2sb, t2_shard)

        if STAGE < 3:
            osb0 = sb.tile([N_GRAPHS, 1], dt.float32, tag="osb", name="osb0")
            nc.vector.memset(osb0[:, :], 0.0)
            nc.sync.dma_start(out=out_ext, in_=osb0[:, :])

        if STAGE >= 3:
            nc.gpsimd.collective_compute(
                "AllGather", mybir.AluOpType.bypass, replica_groups=rg,
                ins=[t2_shard[:, :]], outs=[table2_t[:, :]])
            psC = ctx.enter_context(tc.tile_pool(name="psC", bufs=1, space="PSUM"))
            pa, pb = edge_layer(nc, tc, sb, psA, None, psC, 2,
                                kws[:NW_LIM], K0s,
                                table2_t, idx16, maskT_in, wm0_in, er2_sb, None,
                                ident, iota_row, gidc, invc, None, None)

            # ---------------- pooling reduce + head ----------------
            pasb = sb.tile([128, N_GRAPHS], dt.float32, tag="pasb", name="pasb")
            nc.vector.tensor_copy(pasb[:, :], pa[:, :])
            pbsb = sb.tile([64, N_GRAPHS], dt.float32, tag="pbsb", name="pbsb")
            nc.vector.tensor_copy(pbsb[:, :], pb[:, :])
            nc.sync.dma_start(out=pool_loc[0:128, :], in_=pasb[:, :])
            nc.sync.dma_start(out=pool_loc[128:192, :], in_=pbsb[:, :])
            pra = sb.tile([128, N_GRAPHS], dt.float32, tag="pra", name="pra")
            prb = sb.tile([65, N_GRAPHS], dt.float32, tag="prb", name="prb")
            nc.gpsimd.collective_compute(
                "AllReduce", mybir.AluOpType.add, replica_groups=rg,
                ins=[pool_loc[:, :]], outs=[pool_red_t[:, :]])
            nc.sync.dma_start(out=pra[:, :], in_=pool_red_t[0:128, :])
            nc.sync.dma_start(out=prb[:64, :], in_=pool_red_t[128:192, :])
            nc.vector.memset(prb[64:, :], 1.0)

            u1 = psA.tile([64, N_GRAPHS], dt.float32, tag="erp", name="u1")
            nc.tensor.matmul(u1[:, :], lhsT=d1a[:, :], rhs=pra[:, :],
                             start=True, stop=False)
            nc.tensor.matmul(u1[:, :], lhsT=d1b[:, :], rhs=prb[:, :],
                             start=False, stop=True)
            h1 = sb.tile([65, N_GRAPHS], dt.float32, tag="h1", name="h1")
            nc.scalar.activation(h1[:64, :], u1[:, :],
                                 mybir.ActivationFunctionType.Relu)
            nc.vector.memset(h1[64:, :], 1.0)
            o_ps = psA.tile([N_GRAPHS, 1], dt.float32, tag="agg", name="ops")
            nc.tensor.matmul(o_ps[:, :], lhsT=h1[:, :], rhs=d2w[:, :],
                             start=True, stop=True)
            osb = sb.tile([N_GRAPHS, 1], dt.float32, tag="osb", name="osb")
            nc.vector.tensor_copy(osb[:, :], o_ps[:, :])
            nc.sync.dma_start(out=out_ext, in_=osb[:, :])

    nc.finalize()
    return nc


def edge_layer(nc, tc, sb, psA, psB, psC, layer, kws, K0s,
               tabl, idx16, maskT_in, wm0_in, er_sb, ernext_sb,
               ident, iota_row, gidc, invc, w2sb, t2_shard):
    R = R1 if layer == 1 else R2
    D = D1 if layer == 1 else D2
    B = B1 if layer == 1 else B2
    ZC = ZC1 if layer == 1 else ZC2
    EL = EL1 if layer == 1 else EL2
    ERo = ER1 if layer == 1 else ER2
    XC = X1C if layer == 1 else X2C
    KMAX = max(a + b for a, b in kws)
    eps = 1e-16
    WOP = int(os.environ.get("K_WOP", "9"))

    if layer == 2:
        pa = psC.tile([128, N_GRAPHS], dt.float32, tag="poolA", name="poolA")
        pb = psC.tile([64, N_GRAPHS], dt.float32, tag="poolB", name="poolB")
    gq = 0

    for w in range(len(kws)):
        kA, kB = kws[w]
        k = kA + kB
        K0 = int(K0s[w])
        n0 = w * 128
        nw = min(128, NPC - n0)

        # batched gathers: slot i of window -> zg[i%128, i//128, :]
        # lo section (chunks 0..kA): src < NLO; hi section: src-NLO from offset view
        zg = sb.tile([128, KMAX, R], dt.bfloat16, tag="zg", bufs=3, name="zg")
        GMAX = 8  # >1024 descriptors per dma_gather wedges the device
        for base, koff, ksec in ((0, 0, kA), (NLO, kA, kB)):
            for g0 in range(0, ksec, GMAX):
                gk = min(GMAX, ksec - g0)
                c0 = koff + g0
                nc.gpsimd.dma_gather(
                    out_ap=zg[:, c0:c0 + gk, :],
                    in_ap=tabl[base:, :] if base else tabl[:, :],
                    idxs_ap=idx16[:, 8 * (K0 + c0):8 * (K0 + c0 + gk)],
                    num_idxs=gk * 128, num_idxs_reg=gk * 128, elem_size=R,
                    queue_num=gq % 4)
                gq += 1

        maskT = sb.tile([128, KMAX * 128], tabl.dtype if False else maskT_in.dtype, tag="maskT", name="maskT")
        nc.sync.dma_start(out=maskT[:, 0:k * 128],
                          in_=maskT_in[:, 128 * K0:128 * (K0 + k)])
        wm0 = sb.tile([128, KMAX * 128], maskT_in.dtype, tag="wm0", name="wm0")
        nc.sync.dma_start(out=wm0[:, 0:k * 128],
                          in_=wm0_in[:, 128 * K0:128 * (K0 + k)])

        # er expand node->edge: erp[p,c,j] = er[dst(edge (c,p)), j]
        erp = psA.tile([128, KMAX, 4], dt.float32, tag="erp", name="erp")
        if WOP >= 2:
            for c in range(k):
                nc.tensor.matmul(erp[:, c, 0:4],
                                 lhsT=maskT[:, c * 128:(c + 1) * 128],
                                 rhs=er_sb[:, 4 * w:4 * w + 4],
                                 start=True, stop=True)
        # e = el + er ; w = exp(lrelu(e))
        wb = sb.tile([128, KMAX, 4], dt.float32, tag="wb", name="wb")
        if WOP >= 3:
            ebuf = sb.tile([128, KMAX, 4], dt.float32, tag="ebuf", name="ebuf")
            nc.vector.tensor_tensor(out=ebuf[:, 0:k, :], in0=erp[:, 0:k, :],
                                    in1=zg[:, 0:k, EL:EL + 4],
                                    op=mybir.AluOpType.add)
            esc = sb.tile([128, KMAX, 4], dt.float32, tag="esc", name="esc")
            nc.vector.tensor_scalar(out=esc[:, 0:k, :], in0=ebuf[:, 0:k, :],
                                    scalar1=NEG, scalar2=None,
                                    op0=mybir.AluOpType.mult)
            elr = sb.tile([128, KMAX, 4], dt.float32, tag="elr", name="elr")
            nc.vector.tensor_tensor(out=elr[:, 0:k, :], in0=ebuf[:, 0:k, :],
                                    in1=esc[:, 0:k, :], op=mybir.AluOpType.max)
            nc.scalar.activation(wb[:, 0:k, :], elr[:, 0:k, :],
                                 mybir.ActivationFunctionType.Exp)
        else:
            nc.vector.memset(wb[:, :, :], 1.0)

        # scale gathered z blocks (incl ones col) by w, in place
        if WOP >= 4:
            for c in range(k):
                for h in range(H):
                    nc.vector.tensor_scalar(
                        out=zg[:, c, h * B:(h + 1) * B],
                        in0=zg[:, c, h * B:(h + 1) * B],
                        scalar1=wb[:, c, h:h + 1], scalar2=None,
                        op0=mybir.AluOpType.mult)
        # aggregation
        agg = psA.tile([128, ZC], dt.float32, tag="agg", name="agg")
        if WOP >= 5:
            for c in range(k):
                nc.tensor.matmul(agg[:, :], lhsT=wm0[:, c * 128:(c + 1) * 128],
                                 rhs=zg[:, c, 0:ZC],
                                 start=(c == 0), stop=(c == k - 1))

        xsb = sb.tile([128, XC], dt.bfloat16, tag="xsb", name="xsb")
        if WOP >= 6:
            # epilogue: x = relu(agg_z / s) (* 1/cnt for layer 2)
            seps = sb.tile([128, H], dt.float32, tag="seps", name="seps")
            for h in range(H):
                nc.vector.tensor_scalar(out=seps[:, h:h + 1],
                                        in0=agg[:, h * B + D:h * B + D + 1],
                                        scalar1=eps, scalar2=None,
                                        op0=mybir.AluOpType.add)
            invs = sb.tile([128, H], dt.float32, tag="invs", name="invs")
            nc.vector.reciprocal(invs[:, :], seps[:, :])
            if layer == 2:
                nc.vector.tensor_scalar(out=invs[:, :], in0=invs[:, :],
                                        scalar1=invc[:, w:w + 1], scalar2=None,
                                        op0=mybir.AluOpType.mult)
            for h in range(H):
                nc.scalar.activation(xsb[:, h * D:(h + 1) * D],
                                     agg[:, h * B:h * B + D],
                                     mybir.ActivationFunctionType.Relu,
                                     scale=invs[:, h:h + 1])
        else:
            nc.vector.memset(xsb[:, :], 0.01)

        if layer == 1:
            # transpose x1 -> z2 rows -> t2 shard (+ er2 extraction)
            xtp = psB.tile([X1C, 128], dt.bfloat16, tag="xtp", name="xtp")
            nc.tensor.transpose(xtp[:, :], xsb[:, :], ident[:])
            xta = sb.tile([X1C + 1, 128], dt.bfloat16, tag="xta", name="xta")
            nc.vector.tensor_copy(xta[:X1C, :], xtp[:, :])
            nc.vector.memset(xta[X1C:, :], 1.0)
            z2p = psB.tile([128, R2], dt.float32, tag="zbig", name="z2p")
            nc.tensor.matmul(z2p[:, :], lhsT=xta[:, :], rhs=w2sb[:],
                             start=True, stop=True)
            z2b = sb.tile([128, R2], dt.bfloat16, tag="z2b", name="z2b")
            nc.vector.tensor_copy(z2b[:, :], z2p[:, :])
            nc.vector.tensor_copy(ernext_sb[:nw, 4 * w:4 * w + 3],
                                  z2p[:nw, ER2:ER2 + 3])
            nc.sync.dma_start(out=t2_shard[n0:n0 + nw, :], in_=z2b[:nw, :])
        else:
            pm = sb.tile([128, N_GRAPHS], dt.bfloat16, tag="pm", name="pm")
            nc.vector.tensor_scalar(out=pm[:], in0=iota_row[:],
                                    scalar1=gidc[:, w:w + 1], scalar2=None,
                                    op0=mybir.AluOpType.is_equal)
            nc.tensor.matmul(pa[:, :], lhsT=xsb[:, 0:128], rhs=pm[:],
                             start=(w == 0), stop=(w == len(kws) - 1))
            nc.tensor.matmul(pb[:, :], lhsT=xsb[:, 128:192], rhs=pm[:],
                             start=(w == 0), stop=(w == len(kws) - 1))
    if layer == 2:
        return pa, pb


# ======================= host side =======================

def _prep(feature, src, dst, graph_ids, W1, al1, ar1, W2, al2, ar2,
          d1_w, d1_b, d2_w, d2_b):
    feature = np.asarray(feature, np.float32)
    src = np.asarray(src, np.int64)
    dst = np.asarray(dst, np.int64)
    graph_ids = np.asarray(graph_ids, np.int64)

    order = np.argsort(dst, kind="stable")
    src_s = src[order].astype(np.int32)
    dst_s = dst[order].astype(np.int32)

    cnts = np.bincount(graph_ids, minlength=N_GRAPHS).astype(np.float32)
    cnts = np.maximum(cnts, 1.0)
    node_inv = (1.0 / cnts)[graph_ids]

    # window boundaries; per-window lo/hi chunk counts = max over cores
    percore = []
    kAs = np.zeros(NWIN, np.int64)
    kBs = np.zeros(NWIN, np.int64)
    for r in range(NC):
        wins = []
        for w in range(NWIN):
            lo = r * NPC + w * 128
            hi = min(r * NPC + NPC, lo + 128)
            e0 = np.searchsorted(dst_s, lo, side="left")
            e1 = np.searchsorted(dst_s, hi, side="left")
            m = int((src_s[e0:e1] < NLO).sum())
            q = (e1 - e0) - m
            wins.append((lo, hi, e0, e1))
            kAs[w] = max(kAs[w], _ceil(m, 128))
            kBs[w] = max(kBs[w], _ceil(q, 128))
        percore.append(wins)
    kAs = np.maximum(kAs, 1)  # keep >=1 chunk so every window aggregates
    kws = tuple((int(a), int(b)) for a, b in zip(kAs, kBs))
    ktot = [a + b for a, b in kws]
    TCH = sum(ktot)
    K0s = np.concatenate([[0], np.cumsum(ktot)]).astype(int)
    ECOLS = TCH * 128

    # weight prep
    W1 = np.asarray(W1, np.float32); W2 = np.asarray(W2, np.float32)
    al1 = np.asarray(al1, np.float32); ar1 = np.asarray(ar1, np.float32)
    al2 = np.asarray(al2, np.float32); ar2 = np.asarray(ar2, np.float32)

    def wcat(W, al, ar, D, B, ZC, R):
        F = W.shape[0]
        A_l = np.zeros((H * D, H), np.float32)
        A_r = np.zeros((H * D, H), np.float32)
        for h in range(H):
            A_l[h * D:(h + 1) * D, h] = al[h]
            A_r[h * D:(h + 1) * D, h] = ar[h]
        Wl = W @ A_l
        Wr = W @ A_r
        out = np.zeros((F + 1, R), np.float32)
        for h in range(H):
            out[:F, h * B:h * B + D] = W[:, h * D:(h + 1) * D]
            out[F, h * B + D] = 1.0          # ones column
        out[:F, ZC:ZC + 3] = Wl
        out[:F, ZC + 3:ZC + 6] = Wr
        return out

    wcat1 = wcat(W1, al1, ar1, D1, B1, ZC1, R1)
    wcat2 = wcat(W2, al2, ar2, D2, B2, ZC2, R2).astype(ml_dtypes.bfloat16)

    d1_w = np.asarray(d1_w, np.float32); d1_b = np.asarray(d1_b, np.float32)
    d2_w = np.asarray(d2_w, np.float32); d2_b = np.asarray(d2_b, np.float32)
    d1a = d1_w[0:128, :].copy()
    d1b = np.vstack([d1_w[128:192, :], d1_b[None, :]]).astype(np.float32)
    d2a = np.vstack([d2_w, d2_b[None, :]]).astype(np.float32)

    featT_all = feature.T.astype(np.float32)
    M8 = int(os.environ.get("K_MASK8", "1"))
    ONE = np.uint8(0x38) if M8 else np.uint16(0x3F80)
    mnp = np.uint8 if M8 else np.uint16
    mview = ml_dtypes.float8_e4m3 if M8 else ml_dtypes.bfloat16

    in_maps = []
    for r in range(NC):
        idx16 = np.zeros((16, 8 * TCH), np.int16)
        maskT = np.zeros((128, ECOLS), mnp)
        wm0 = np.zeros((128, ECOLS), mnp)
        for w, (lo, hi, e0, e1) in enumerate(percore[r]):
            kA, kB = kws[w]
            K0 = int(K0s[w])
            es = src_s[e0:e1]
            ed = dst_s[e0:e1] - lo
            is_lo = es < NLO
            for sec, (sel, base, koff, ksec) in enumerate([
                    (is_lo, 0, 0, kA), (~is_lo, NLO, kA, kB)]):
                s = es[sel] - base
                dv = ed[sel].astype(np.int64)
                cnt = len(s)
                nsl = ksec * 128
                if ksec == 0:
                    assert cnt == 0
                    continue
                sv = np.zeros(nsl, np.int16)
                sv[:cnt] = s.astype(np.int16)
                c0 = 8 * (K0 + koff)
                idx16[:, c0:c0 + 8 * ksec] = sv.reshape(-1, 16).T
                i = np.arange(cnt)
                # slot i -> chunk (K0+koff+i//128), partition i%128
                maskT[dv, 128 * (K0 + koff) + i] = ONE
                wm0[i % 128, 128 * (K0 + koff) + (i // 128) * 128 + dv] = ONE
        gidc = np.full((128, NWIN), -1.0, np.float32)
        invc = np.zeros((128, NWIN), np.float32)
        for w in range(NWIN):
            lo, hi, _, _ = percore[r][w]
            nw = hi - lo
            gidc[:nw, w] = graph_ids[lo:hi].astype(np.float32)
            invc[:nw, w] = node_inv[lo:hi]
        ft = np.vstack([featT_all[:, r * NPC:(r + 1) * NPC],
                        np.ones((1, NPC), np.float32)])
        in_maps.append({
            "featT": ft, "wcat1": wcat1, "wcat2": wcat2,
            "idx16": np.tile(idx16, (8, 1)),
            "maskT": maskT.view(mview),
            "wm0": wm0.view(mview),
            "gidc": gidc, "invc": invc,
            "d1a": d1a, "d1b": d1b, "d2": d2a,
            "ident": np.eye(128, dtype=ml_dtypes.bfloat16),
            "iota_row": np.tile(np.arange(128, dtype=ml_dtypes.bfloat16)[None, :],
                                (128, 1)),
        })
    return in_maps, kws


def kernel(**inputs):
    in_maps, kws = _prep(**inputs)
    if kws not in _CACHE:
        _CACHE[kws] = build_program(kws)
    nc = _CACHE[kws]
    res = run_bass_kernel_spmd(nc, in_maps, list(range(NC)))
    return res.results[0]["out"]

